# revision 2
# baseline (speedup 1.0000x reference)
"""CostVolume2D Trainium2 Bass kernel, v2 (bf16 multi-engine pipeline).

cost[b,h,w,d] = sum_c |feat_l[b,h,w,c] - feat_r[b,h,w-d,c]|, zero-padded left.

Pure data-parallel over batch B=8 across 8 cores. Per core the work is
12 disparities x 4.19M elems of sub + abs + 32-way c-reduction. Engine
assignment (all rates per-core):
  - DVE (bottleneck): bf16 tensor_sub at 2x mode (2 elem/cyc/lane) and the
    c-reduction as a pairwise bf16 tensor_add tree (levels 16,8,4,2 at 2x,
    final level 1x to f32). This halves DVE cost vs f32 sub (1x) +
    tensor_reduce (always 1x): ~16.4k cyc vs ~32.8k per (d, chunk).
  - abs: offloaded off the DVE critical path; per-disparity either ScalarE
    activation(Abs) or GPSIMD tensor_scalar bitwise-AND 0x7fff on the
    bitcast uint16 view (1-input ops run near line rate on both).
  - ScalarE also: f32->bf16 input conversion, CT->CO d-interleave copies.
  - DMA: full-tile contiguous loads/stores, overlapped with compute.

Processing unit: chunk = (slab of 128 h-rows) x (256 w-cols); 4 chunks/core.
FRb keeps a 12-column zero pad on the left of each w-chunk so disparity
shifts are plain free-dim offsets and the w<d boundary reads zeros.
"""

import numpy as np

import concourse.bass as bass
import concourse.mybir as mybir
from concourse.instruction_name_ordered_set import InstructionNameOrderedSet
from concourse.bass_utils import run_bass_kernel_spmd

B, H, W, C, D = 8, 256, 512, 32, 12
N_CORES = 8
P = 128          # partitions per slab
WCH = 256        # w-chunk
NJ = W // WCH    # w-chunks per slab (2)
NS = H // P      # slabs (2)
PAD = D * C      # 384 elems of left zero-pad in FRb
FCH = WCH * C    # 8192: elems per partition per chunk (l side)
RCH = PAD + FCH  # 8576: r side with pad

F32 = mybir.dt.float32
BF16 = mybir.dt.bfloat16
U16 = mybir.dt.uint16

# abs owner per disparity: "scal" (activation Abs) or "dve" (uint16 AND at
# 4x on the vector engine). GPSIMD rejects TensorScalarPtr, so it does the
# f32->bf16 input conversions (tensor_copy) instead of abs.
ABS_OWNERS = ("scal", "scal", "scal", "dve", "scal", "scal",
              "scal", "dve", "scal", "scal", "scal", "dve")

_NC_CACHE = {}


def build_nc(reps=1, abs_owners=ABS_OWNERS):
    G = NS * NJ * reps  # total chunk-units
    nc = bass.Bass()
    fl = nc.dram_tensor("feat_l", [H, W * C], F32, kind="ExternalInput")
    fr = nc.dram_tensor("feat_r", [H, W * C], F32, kind="ExternalInput")
    cost = nc.dram_tensor("cost", [H, W * D], F32, kind="ExternalOutput")

    # per-(g,d) abs owner bookkeeping: cumulative per-owner counts
    owner_of = {}
    cum = {"gp": 0, "scal": 0, "dve": 0}
    cum_at = {}  # (g, d) -> (owner, count_after_this_op)
    for g in range(G):
        for d in range(D):
            o = abs_owners[d]
            owner_of[(g, d)] = o
            cum[o] += 1
            cum_at[(g, d)] = (o, cum[o])

    def loads_done(g):   # dma_sem value once chunk g's two loads completed
        return 16 * (3 * g + 2)

    def store_done(g):   # dma_sem value once chunk g's CO store completed
        return 48 * (g + 1)

    from contextlib import ExitStack

    with ExitStack() as stack:
        en = stack.enter_context
        FL32 = en(nc.sbuf_tensor([P, FCH], F32))
        FR32 = en(nc.sbuf_tensor([P, RCH], F32))
        FLb0 = en(nc.sbuf_tensor([P, FCH], BF16))
        FLb1 = en(nc.sbuf_tensor([P, FCH], BF16))
        FRb0 = en(nc.sbuf_tensor([P, RCH], BF16))
        FRb1 = en(nc.sbuf_tensor([P, RCH], BF16))
        DIFF0 = en(nc.sbuf_tensor([P, FCH], BF16))
        DIFF1 = en(nc.sbuf_tensor([P, FCH], BF16))
        T1 = en(nc.sbuf_tensor([P, WCH * 16], BF16))
        T2 = en(nc.sbuf_tensor([P, WCH * 8], BF16))
        T3 = en(nc.sbuf_tensor([P, WCH * 4], BF16))
        T4 = en(nc.sbuf_tensor([P, WCH * 2], BF16))
        CT0 = en(nc.sbuf_tensor([P, WCH], F32))
        CT1 = en(nc.sbuf_tensor([P, WCH], F32))
        CO = en(nc.sbuf_tensor([P, WCH * D], F32))
        MASK = en(nc.sbuf_tensor([P, 1], U16))
        dma_sem = en(nc.semaphore("dma_sem"))
        conv_sem = en(nc.semaphore("conv_sem"))
        sub_sem = en(nc.semaphore("sub_sem"))
        absg_sem = en(nc.semaphore("absg_sem"))
        abss_sem = en(nc.semaphore("abss_sem"))
        ct_sem = en(nc.semaphore("ct_sem"))
        act_sem = en(nc.semaphore("act_sem"))
        block = en(nc.Block())
        FLb = [FLb0, FLb1]
        FRb = [FRb0, FRb1]
        DIFF = [DIFF0, DIFF1]
        CTS = [CT0, CT1]
        abs_sems = {"gp": absg_sem, "scal": abss_sem}

        def chunk_params(g):
            k = g % (NS * NJ)
            s, j = k // NJ, k % NJ
            return s, j, g % 2  # slab, w-chunk, buffer parity (== j parity)

        @block.sync
        def _(sync):
            for g in range(G):
                s, j, b = chunk_params(g)
                if g >= 1:
                    sync.wait_ge(conv_sem, g)  # FL32/FR32 free after conv g-1
                sync.dma_start(
                    out=FL32[:, :],
                    in_=fl[s * P : (s + 1) * P, j * FCH : (j + 1) * FCH],
                ).then_inc(dma_sem, 16)
                if j == 0:
                    sync.dma_start(
                        out=FR32[:, PAD:],
                        in_=fr[s * P : (s + 1) * P, 0:FCH],
                    ).then_inc(dma_sem, 16)
                else:
                    sync.dma_start(
                        out=FR32[:, :],
                        in_=fr[s * P : (s + 1) * P, j * FCH - PAD : (j + 1) * FCH],
                    ).then_inc(dma_sem, 16)
                if g >= 1:
                    sp, jp, _ = chunk_params(g - 1)
                    sync.wait_ge(act_sem, D * g)  # chunk g-1 fully interleaved
                    sync.dma_start(
                        out=cost[sp * P : (sp + 1) * P, jp * WCH * D : (jp + 1) * WCH * D],
                        in_=CO[:, :],
                    ).then_inc(dma_sem, 16)
            sp, jp, _ = chunk_params(G - 1)
            sync.wait_ge(act_sem, D * G)
            sync.dma_start(
                out=cost[sp * P : (sp + 1) * P, jp * WCH * D : (jp + 1) * WCH * D],
                in_=CO[:, :],
            ).then_inc(dma_sem, 16)
            sync.wait_ge(dma_sem, 48 * G)
            for sem in (dma_sem, conv_sem, sub_sem, absg_sem, abss_sem,
                        ct_sem, act_sem):
                sync.sem_clear(sem)

        @block.vector
        def _(vector):
            prev = [None]

            def chain(inst):
                if prev[0] is not None:
                    deps = InstructionNameOrderedSet()
                    deps.add(prev[0].ins.name)
                    inst.ins.add_nosync_dependencies_from(deps)
                prev[0] = inst
                return inst

            # zero pad of FRb0 (j==0 buffer), once; persists across chunks
            chain(vector.memset(FRb0[:, 0:PAD], 0.0))
            chain(vector.memset(MASK[:, :], 0x7FFF))

            def do_sub(g, d):
                _, _, b = chunk_params(g)
                # DIFF[d%2] WAR: abs+tree of (g,d-2)/(g-1,10+d) must be done.
                # tree is same-engine (FIFO); abs is cross-engine -> wait.
                gp_, dp_ = (g, d - 2) if d >= 2 else (g - 1, 10 + d)
                if gp_ >= 0:
                    o, cnt = cum_at[(gp_, dp_)]
                    if o == "dve":
                        pass  # same engine, FIFO
                    else:
                        chain(vector.wait_ge(abs_sems[o], cnt))
                if d == 0:
                    chain(vector.wait_ge(conv_sem, g + 1))
                r0 = (D - d) * C
                chain(
                    vector.tensor_sub(
                        DIFF[d % 2][:, :],
                        FLb[b][:, :],
                        FRb[b][:, r0 : r0 + FCH],
                    )
                ).then_inc(sub_sem, 1)

            def do_tree(g, d):
                o, cnt = cum_at[(g, d)]
                if o == "dve":
                    du = DIFF[d % 2][:, :].bitcast(U16)
                    chain(
                        vector.tensor_scalar(
                            du, du, MASK[:, :], None,
                            op0=mybir.AluOpType.bitwise_and,
                        )
                    )
                else:
                    chain(vector.wait_ge(abs_sems[o], cnt))
                d3 = DIFF[d % 2][:, :].rearrange("p (w c) -> p w c", c=C)
                t1 = T1[:, :].rearrange("p (w c) -> p w c", c=16)
                t2 = T2[:, :].rearrange("p (w c) -> p w c", c=8)
                t3 = T3[:, :].rearrange("p (w c) -> p w c", c=4)
                t4 = T4[:, :].rearrange("p (w c) -> p w c", c=2)
                chain(vector.tensor_add(T1[:, :].rearrange("p (w c) -> p w c", c=16),
                                        d3[:, :, 0:16], d3[:, :, 16:32]))
                chain(vector.tensor_add(t2, t1[:, :, 0:8], t1[:, :, 8:16]))
                chain(vector.tensor_add(t3, t2[:, :, 0:4], t2[:, :, 4:8]))
                chain(vector.tensor_add(t4, t3[:, :, 0:2], t3[:, :, 2:4]))
                # CT ping-pong WAR vs interleave of (g,d-2)/(g-1,10+d)
                w_act = D * g + d - 1
                if w_act > 0:
                    chain(vector.wait_ge(act_sem, w_act))
                chain(
                    vector.tensor_add(
                        CTS[d % 2][:, :], t4[:, :, 0:1], t4[:, :, 1:2]
                    )
                ).then_inc(ct_sem, 1)

            for g in range(G):
                # software-pipelined: sub(d+1) runs while abs(d) is in flight
                do_sub(g, 0)
                for d in range(D):
                    if d + 1 < D:
                        do_sub(g, d + 1)
                    do_tree(g, d)

        @block.gpsimd
        def _(gpsimd):
            prevg = [None]

            def chaing(inst):
                if prevg[0] is not None:
                    deps = InstructionNameOrderedSet()
                    deps.add(prevg[0].ins.name)
                    inst.ins.add_nosync_dependencies_from(deps)
                prevg[0] = inst
                return inst

            def convs(g):
                _, j, b = chunk_params(g)
                chaing(gpsimd.wait_ge(dma_sem, loads_done(g)))
                if g >= 1:
                    # FLb/FRb[b] WAR: subs of chunk g-2 must be done
                    chaing(gpsimd.wait_ge(sub_sem, D * (g - 1)))
                chaing(gpsimd.tensor_copy(out=FLb[b][:, :], in_=FL32[:, :]))
                if j == 0:
                    last = gpsimd.tensor_copy(
                        out=FRb[b][:, PAD:], in_=FR32[:, PAD:]
                    )
                else:
                    last = gpsimd.tensor_copy(out=FRb[b][:, :], in_=FR32[:, :])
                chaing(last).then_inc(conv_sem, 1)

            for g in range(G):
                convs(g)

        @block.scalar
        def _(scalar):
            CO3 = CO[:, :].rearrange("p (w d) -> p w d", d=D)

            for g in range(G):
                # abs for scal-owned disparities, interleaved with CT->CO
                # copies in d order (both gate the DVE, abs most tightly)
                if g >= 1:
                    scalar.wait_ge(dma_sem, store_done(g - 1))  # CO WAR
                for d in range(D):
                    if owner_of[(g, d)] == "scal":
                        scalar.wait_ge(sub_sem, D * g + d + 1)
                        scalar.activation(
                            DIFF[d % 2][:, :], DIFF[d % 2][:, :],
                            mybir.ActivationFunctionType.Abs,
                        ).then_inc(abss_sem, 1)
                    if d >= 2:
                        scalar.wait_ge(ct_sem, D * g + d - 1)
                        scalar.copy(
                            CO3[:, :, d - 2], CTS[d % 2][:, :]
                        ).then_inc(act_sem, 1)
                for d in (10, 11):
                    scalar.wait_ge(ct_sem, D * g + d + 1)
                    scalar.copy(CO3[:, :, d], CTS[d % 2][:, :]).then_inc(
                        act_sem, 1
                    )

    return nc


def _get_nc():
    if "nc" not in _NC_CACHE:
        _NC_CACHE["nc"] = build_nc()
    return _NC_CACHE["nc"]


def _run(feat_l, feat_r, trace=False, nc=None):
    if nc is None:
        nc = _get_nc()
    feat_l = np.asarray(feat_l, dtype=np.float32)
    feat_r = np.asarray(feat_r, dtype=np.float32)
    in_maps = []
    for b in range(B):
        in_maps.append(
            {
                "feat_l": np.ascontiguousarray(feat_l[b].reshape(H, W * C)),
                "feat_r": np.ascontiguousarray(feat_r[b].reshape(H, W * C)),
            }
        )
    res = run_bass_kernel_spmd(nc, in_maps, list(range(N_CORES)), trace=trace)
    out = np.stack(
        [res.results[i]["cost"].reshape(H, W, D) for i in range(B)]
    ).astype(np.float32)
    return out, res


def kernel(feat_l, feat_r):
    out, _ = _run(feat_l, feat_r, trace=False)
    return out


# revision 3
# speedup vs baseline: 1.0150x; 1.0150x over previous
"""CostVolume2D Trainium2 Bass kernel (bf16 multi-engine pipeline).

cost[b,h,w,d] = sum_c |feat_l[b,h,w,c] - feat_r[b,h,w-d,c]|, zero-padded left
(for w < d the reference reduces to sum_c |feat_l|, reproduced exactly by a
12-column zero pad on the left of each feat_r chunk).

Pure data-parallel over batch B=8 across 8 NeuronCores; full inputs in, full
output out, sharding handled inside kernel().

Per-core design (measured 613 us/exec on HW vs 922 us for the f32
sub+tensor_reduce baseline; DVE-only floor of this structure is ~530 us):
  - All heavy math on the DVE in bf16: tensor_sub at 2x mode, abs as a
    uint16 AND 0x7fff on the bitcast diff (tensor_scalar with a [P,1] mask
    AP; an immediate scalar measured slower), and the 32-way c-reduction as
    a pairwise tensor_add tree (16,8,4,2 halves at 2x, final level to f32).
    Tree+AND costs ~7.1k cyc per (d, chunk) vs 12.3k for sub+tensor_reduce
    (tensor_reduce is capped at 1x; fp32 tensor_tensor too).
  - GPSIMD: f32->bf16 input conversion (tensor_copy; 1-input ops run near
    line rate, and it is otherwise idle).
  - ScalarE: CT->CO d-interleave copies (strided writes off the DVE path).
  - Cross-engine handoffs are minimized: measured ~7 us stall per
    sem round-trip made ScalarE-abs slower overall despite fewer DVE
    cycles. CT rotates over 4 buffers so the final tree add only waits on
    an interleave 4 disparities back.
  - Chunk = (128 h-rows) x (256 w-cols), 4 chunks/core; f32 staging tiles
    single-buffered (load waits prior conversion), bf16 tiles
    double-buffered; one contiguous DMA per tensor per chunk.

Timing methodology (see test.py): per-call wall time under axon is seconds
of dispatch noise, so HW time = slope of min wall time over in-NEFF
repetition counts with a cached jitted callable and device-resident inputs.
"""

import numpy as np

import concourse.bass as bass
import concourse.mybir as mybir
from concourse.instruction_name_ordered_set import InstructionNameOrderedSet
from concourse.bass_utils import run_bass_kernel_spmd

B, H, W, C, D = 8, 256, 512, 32, 12
N_CORES = 8
P = 128          # partitions per slab
WCH = 256        # w-chunk
NJ = W // WCH    # w-chunks per slab (2)
NS = H // P      # slabs (2)
PAD = D * C      # 384 elems of left zero-pad in FRb
FCH = WCH * C    # 8192: elems per partition per chunk (l side)
RCH = PAD + FCH  # 8576: r side with pad

F32 = mybir.dt.float32
BF16 = mybir.dt.bfloat16
U16 = mybir.dt.uint16

# abs owner per disparity: "dve" = uint16 AND 0x7fff on the bitcast view,
# issued on the vector engine right before the tree (no cross-engine wait);
# "scal" = ScalarE activation(Abs) in place (cheaper in cycles but each
# cross-engine handoff measured ~7 us of stall, so all-dve wins: 613 us vs
# 692 us on HW). GPSIMD rejects TensorScalarPtr, so it does the f32->bf16
# input conversions (tensor_copy) instead of abs.
ABS_OWNERS = ("dve",) * 12

_NC_CACHE = {}


def build_nc(reps=1, abs_owners=ABS_OWNERS):
    G = NS * NJ * reps  # total chunk-units
    nc = bass.Bass()
    fl = nc.dram_tensor("feat_l", [H, W * C], F32, kind="ExternalInput")
    fr = nc.dram_tensor("feat_r", [H, W * C], F32, kind="ExternalInput")
    cost = nc.dram_tensor("cost", [H, W * D], F32, kind="ExternalOutput")

    # per-(g,d) abs owner bookkeeping: cumulative per-owner counts
    owner_of = {}
    cum = {"gp": 0, "scal": 0, "dve": 0}
    cum_at = {}  # (g, d) -> (owner, count_after_this_op)
    for g in range(G):
        for d in range(D):
            o = abs_owners[d]
            owner_of[(g, d)] = o
            cum[o] += 1
            cum_at[(g, d)] = (o, cum[o])

    def loads_done(g):   # dma_sem value once chunk g's two loads completed
        return 16 * (3 * g + 2)

    def store_done(g):   # dma_sem value once chunk g's CO store completed
        return 48 * (g + 1)

    from contextlib import ExitStack

    with ExitStack() as stack:
        en = stack.enter_context
        FL32 = en(nc.sbuf_tensor([P, FCH], F32))
        FR32 = en(nc.sbuf_tensor([P, RCH], F32))
        FLb0 = en(nc.sbuf_tensor([P, FCH], BF16))
        FLb1 = en(nc.sbuf_tensor([P, FCH], BF16))
        FRb0 = en(nc.sbuf_tensor([P, RCH], BF16))
        FRb1 = en(nc.sbuf_tensor([P, RCH], BF16))
        DIFF0 = en(nc.sbuf_tensor([P, FCH], BF16))
        DIFF1 = en(nc.sbuf_tensor([P, FCH], BF16))
        T1 = en(nc.sbuf_tensor([P, WCH * 16], BF16))
        T2 = en(nc.sbuf_tensor([P, WCH * 8], BF16))
        T3 = en(nc.sbuf_tensor([P, WCH * 4], BF16))
        T4 = en(nc.sbuf_tensor([P, WCH * 2], BF16))
        CT0 = en(nc.sbuf_tensor([P, WCH], F32))
        CT1 = en(nc.sbuf_tensor([P, WCH], F32))
        CT2 = en(nc.sbuf_tensor([P, WCH], F32))
        CT3 = en(nc.sbuf_tensor([P, WCH], F32))
        CO = en(nc.sbuf_tensor([P, WCH * D], F32))
        MASK = en(nc.sbuf_tensor([P, 1], U16))
        dma_sem = en(nc.semaphore("dma_sem"))
        conv_sem = en(nc.semaphore("conv_sem"))
        sub_sem = en(nc.semaphore("sub_sem"))
        absg_sem = en(nc.semaphore("absg_sem"))
        abss_sem = en(nc.semaphore("abss_sem"))
        ct_sem = en(nc.semaphore("ct_sem"))
        act_sem = en(nc.semaphore("act_sem"))
        block = en(nc.Block())
        FLb = [FLb0, FLb1]
        FRb = [FRb0, FRb1]
        DIFF = [DIFF0, DIFF1]
        CTS = [CT0, CT1, CT2, CT3]
        abs_sems = {"gp": absg_sem, "scal": abss_sem}

        def chunk_params(g):
            k = g % (NS * NJ)
            s, j = k // NJ, k % NJ
            return s, j, g % 2  # slab, w-chunk, buffer parity (== j parity)

        @block.sync
        def _(sync):
            for g in range(G):
                s, j, b = chunk_params(g)
                if g >= 1:
                    sync.wait_ge(conv_sem, g)  # FL32/FR32 free after conv g-1
                sync.dma_start(
                    out=FL32[:, :],
                    in_=fl[s * P : (s + 1) * P, j * FCH : (j + 1) * FCH],
                ).then_inc(dma_sem, 16)
                if j == 0:
                    sync.dma_start(
                        out=FR32[:, PAD:],
                        in_=fr[s * P : (s + 1) * P, 0:FCH],
                    ).then_inc(dma_sem, 16)
                else:
                    sync.dma_start(
                        out=FR32[:, :],
                        in_=fr[s * P : (s + 1) * P, j * FCH - PAD : (j + 1) * FCH],
                    ).then_inc(dma_sem, 16)
                if g >= 1:
                    sp, jp, _ = chunk_params(g - 1)
                    sync.wait_ge(act_sem, D * g)  # chunk g-1 fully interleaved
                    sync.dma_start(
                        out=cost[sp * P : (sp + 1) * P, jp * WCH * D : (jp + 1) * WCH * D],
                        in_=CO[:, :],
                    ).then_inc(dma_sem, 16)
            sp, jp, _ = chunk_params(G - 1)
            sync.wait_ge(act_sem, D * G)
            sync.dma_start(
                out=cost[sp * P : (sp + 1) * P, jp * WCH * D : (jp + 1) * WCH * D],
                in_=CO[:, :],
            ).then_inc(dma_sem, 16)
            sync.wait_ge(dma_sem, 48 * G)
            for sem in (dma_sem, conv_sem, sub_sem, absg_sem, abss_sem,
                        ct_sem, act_sem):
                sync.sem_clear(sem)

        @block.vector
        def _(vector):
            prev = [None]

            def chain(inst):
                if prev[0] is not None:
                    deps = InstructionNameOrderedSet()
                    deps.add(prev[0].ins.name)
                    inst.ins.add_nosync_dependencies_from(deps)
                prev[0] = inst
                return inst

            # zero pad of FRb0 (j==0 buffer), once; persists across chunks
            chain(vector.memset(FRb0[:, 0:PAD], 0.0))
            chain(vector.memset(MASK[:, :], 0x7FFF))

            def do_sub(g, d):
                _, _, b = chunk_params(g)
                # DIFF[d%2] WAR: abs+tree of (g,d-2)/(g-1,10+d) must be done.
                # tree is same-engine (FIFO); abs is cross-engine -> wait.
                gp_, dp_ = (g, d - 2) if d >= 2 else (g - 1, 10 + d)
                if gp_ >= 0:
                    o, cnt = cum_at[(gp_, dp_)]
                    if o == "dve":
                        pass  # same engine, FIFO
                    else:
                        chain(vector.wait_ge(abs_sems[o], cnt))
                if d == 0:
                    chain(vector.wait_ge(conv_sem, g + 1))
                r0 = (D - d) * C
                chain(
                    vector.tensor_sub(
                        DIFF[d % 2][:, :],
                        FLb[b][:, :],
                        FRb[b][:, r0 : r0 + FCH],
                    )
                ).then_inc(sub_sem, 1)

            def do_tree(g, d):
                o, cnt = cum_at[(g, d)]
                if o == "dve":
                    du = DIFF[d % 2][:, :].bitcast(U16)
                    chain(
                        vector.tensor_scalar(
                            du, du, MASK[:, :], None,
                            op0=mybir.AluOpType.bitwise_and,
                        )
                    )
                else:
                    chain(vector.wait_ge(abs_sems[o], cnt))
                d3 = DIFF[d % 2][:, :].rearrange("p (w c) -> p w c", c=C)
                t1 = T1[:, :].rearrange("p (w c) -> p w c", c=16)
                t2 = T2[:, :].rearrange("p (w c) -> p w c", c=8)
                t3 = T3[:, :].rearrange("p (w c) -> p w c", c=4)
                t4 = T4[:, :].rearrange("p (w c) -> p w c", c=2)
                chain(vector.tensor_add(T1[:, :].rearrange("p (w c) -> p w c", c=16),
                                        d3[:, :, 0:16], d3[:, :, 16:32]))
                chain(vector.tensor_add(t2, t1[:, :, 0:8], t1[:, :, 8:16]))
                chain(vector.tensor_add(t3, t2[:, :, 0:4], t2[:, :, 4:8]))
                chain(vector.tensor_add(t4, t3[:, :, 0:2], t3[:, :, 2:4]))
                # CT 4-deep rotation: WAR vs interleave of 4 d's ago
                w_act = D * g + d - 3
                if w_act > 0:
                    chain(vector.wait_ge(act_sem, w_act))
                chain(
                    vector.tensor_add(
                        CTS[d % 4][:, :], t4[:, :, 0:1], t4[:, :, 1:2]
                    )
                ).then_inc(ct_sem, 1)

            for g in range(G):
                # software-pipelined: sub(d+1) runs while abs(d) is in flight
                do_sub(g, 0)
                for d in range(D):
                    if d + 1 < D:
                        do_sub(g, d + 1)
                    do_tree(g, d)

        @block.gpsimd
        def _(gpsimd):
            prevg = [None]

            def chaing(inst):
                if prevg[0] is not None:
                    deps = InstructionNameOrderedSet()
                    deps.add(prevg[0].ins.name)
                    inst.ins.add_nosync_dependencies_from(deps)
                prevg[0] = inst
                return inst

            def convs(g):
                _, j, b = chunk_params(g)
                chaing(gpsimd.wait_ge(dma_sem, loads_done(g)))
                if g >= 1:
                    # FLb/FRb[b] WAR: subs of chunk g-2 must be done
                    chaing(gpsimd.wait_ge(sub_sem, D * (g - 1)))
                chaing(gpsimd.tensor_copy(out=FLb[b][:, :], in_=FL32[:, :]))
                if j == 0:
                    last = gpsimd.tensor_copy(
                        out=FRb[b][:, PAD:], in_=FR32[:, PAD:]
                    )
                else:
                    last = gpsimd.tensor_copy(out=FRb[b][:, :], in_=FR32[:, :])
                chaing(last).then_inc(conv_sem, 1)

            for g in range(G):
                convs(g)

        @block.scalar
        def _(scalar):
            CO3 = CO[:, :].rearrange("p (w d) -> p w d", d=D)

            for g in range(G):
                # abs for scal-owned disparities, interleaved with CT->CO
                # copies in d order (both gate the DVE, abs most tightly)
                if g >= 1:
                    scalar.wait_ge(dma_sem, store_done(g - 1))  # CO WAR
                for d in range(D):
                    if owner_of[(g, d)] == "scal":
                        scalar.wait_ge(sub_sem, D * g + d + 1)
                        scalar.activation(
                            DIFF[d % 2][:, :], DIFF[d % 2][:, :],
                            mybir.ActivationFunctionType.Abs,
                        ).then_inc(abss_sem, 1)
                    if d >= 2:
                        scalar.wait_ge(ct_sem, D * g + d - 1)
                        scalar.copy(
                            CO3[:, :, d - 2], CTS[(d - 2) % 4][:, :]
                        ).then_inc(act_sem, 1)
                for d in (10, 11):
                    scalar.wait_ge(ct_sem, D * g + d + 1)
                    scalar.copy(CO3[:, :, d], CTS[d % 4][:, :]).then_inc(
                        act_sem, 1
                    )

    return nc


def _get_nc():
    if "nc" not in _NC_CACHE:
        _NC_CACHE["nc"] = build_nc()
    return _NC_CACHE["nc"]


def _run(feat_l, feat_r, trace=False, nc=None):
    if nc is None:
        nc = _get_nc()
    feat_l = np.asarray(feat_l, dtype=np.float32)
    feat_r = np.asarray(feat_r, dtype=np.float32)
    in_maps = []
    for b in range(B):
        in_maps.append(
            {
                "feat_l": np.ascontiguousarray(feat_l[b].reshape(H, W * C)),
                "feat_r": np.ascontiguousarray(feat_r[b].reshape(H, W * C)),
            }
        )
    res = run_bass_kernel_spmd(nc, in_maps, list(range(N_CORES)), trace=trace)
    out = np.stack(
        [res.results[i]["cost"].reshape(H, W, D) for i in range(B)]
    ).astype(np.float32)
    return out, res


def kernel(feat_l, feat_r):
    out, _ = _run(feat_l, feat_r, trace=False)
    return out
